# revision 1
# baseline (speedup 1.0000x reference)
import sys, os
import numpy as np

sys.path.insert(0, '/opt/trn_rl_repo')

N = 50000; E = 800000; IN = 128; HID = 64; H = 4; G = 5; K = 3; OUT = 1
NC = 8
SH = N // NC              # 6250 nodes per core
NP_PAD = 6656             # padded per-core nodes (52 x 128)
NBLK = NP_PAD // 128      # 52 GAT blocks per core
TMAX = 20                 # edge tiles (x128) per block, static capacity
GRP = 10                  # gather sub-group (tiles per group)
CH = 512                  # KAN node chunk
NCH = NP_PAD // CH        # 13
NM = 11                   # psi shifts m = 0..10
NB = 1 + 3 * NM           # phi basis dim: const + {psi, psi^2, psi^3}
HGRID = 2.0 / G           # 0.4
ULO = -1.0 - K * HGRID    # -2.2
USC = 1.0 / HGRID         # 2.5
UBI = -ULO / HGRID        # 5.5
NXCH = (N + 127) // 128   # 391 chunks for xp build


def _grid():
    return (np.arange(-K, G + K + 1, dtype=np.float64) * HGRID - 1.0)


def _b_splines_np(x):
    # x: [n, i] float64 -> [n, i, G+K]
    g = _grid()
    xg = x[..., None]
    b = ((xg >= g[:-1]) & (xg < g[1:])).astype(np.float64)
    for p in range(1, K + 1):
        b = ((xg - g[:-(p + 1)]) / (g[p:-1] - g[:-(p + 1)])) * b[..., :-1] \
          + ((g[p + 1:] - xg) / (g[p + 1:] - g[1:-p])) * b[..., 1:]
    return b


def _phi_np(u):
    cols = [np.ones_like(u)]
    for m in range(NM):
        v = np.maximum(u - m, 0.0)
        psi = np.maximum(1.0 - v, 0.0)
        cols += [psi, psi * psi, psi * psi * psi]
    return np.stack(cols, axis=1)


def _fit_M():
    # b_k(x) = phi(u(x)) @ M,  M: [NB, 8]
    u = np.linspace(-6.0, 18.0, 6001)
    x = (u - UBI) / USC
    B = _b_splines_np(x[:, None]).reshape(-1, G + K)
    Phi = _phi_np(u)
    M, res, _, _ = np.linalg.lstsq(Phi, B, rcond=None)
    err = np.abs(Phi @ M - B).max()
    return M, err


def _silu(x):
    return x / (1.0 + np.exp(-x))


def _host_gat(x, ei, W, a_src, a_dst, bias):
    xp = (x @ W.T).reshape(N, H, HID)
    as_ = (xp * a_src).sum(-1).astype(np.float32)
    ad_ = (xp * a_dst).sum(-1).astype(np.float32)
    loops = np.arange(N, dtype=np.int64)
    src = np.concatenate([ei[0].astype(np.int64), loops])
    dst = np.concatenate([ei[1].astype(np.int64), loops])
    order = np.argsort(dst, kind='stable')
    src = src[order]; dst = dst[order]
    e = as_[src] + ad_[dst]
    e = np.where(e > 0, e, np.float32(0.2) * e)
    starts = np.searchsorted(dst, np.arange(N, dtype=np.int64))
    m = np.maximum.reduceat(e, starts, axis=0)
    ex = np.exp(e - m[dst])
    s = np.add.reduceat(ex, starts, axis=0)
    alpha = ex / s[dst]
    out = np.empty((N, H, HID), np.float32)
    for h in range(H):
        tmp = xp[src, h, :] * alpha[:, h:h + 1]
        out[:, h, :] = np.add.reduceat(tmp, starts, axis=0)
    return out.mean(axis=1) + bias


def _fold_layer(base_w, spline_w, scaler, M, fin_pad):
    o, fin = base_w.shape
    A = (spline_w * scaler[..., None]).astype(np.float64)   # [o, fin, 8]
    At = np.einsum('oik,kf->oif', A, M.T)                   # [o, fin, NB]
    bias = At[:, :, 0].sum(axis=1).astype(np.float32)
    Asp = At[:, :, 1:]                                      # [o, fin, 33]
    rows = 128 if fin_pad >= 128 else fin_pad
    nft = fin_pad // rows
    blocks = []
    for f in range(nft):                                     # silu blocks
        blk = np.zeros((rows, o), np.float32)
        lo = f * rows; hi = min(fin, lo + rows)
        if hi > lo:
            blk[:hi - lo, :] = base_w[:, lo:hi].T
        blocks.append(blk)
    for f in range(nft):
        lo = f * rows; hi = min(fin, lo + rows)
        for m in range(NM):
            for p in range(3):
                blk = np.zeros((rows, o), np.float32)
                if hi > lo:
                    blk[:hi - lo, :] = Asp[:, lo:hi, m * 3 + p].T
                blocks.append(blk)
    return np.stack(blocks).astype(np.float32), bias


def _host_kan(xc, weights):
    h = xc.astype(np.float64)
    for li, (bw, sw, sc) in enumerate(weights):
        b = _b_splines_np(h)
        spl = np.einsum('nik,oik->no', b, (sw * sc[..., None]).astype(np.float64))
        h = _silu(h) @ bw.T + spl
        if li == 1:
            h = np.maximum(h, 0.0)
    return h.astype(np.float32)


_BASS_CACHE = {}
LAYER_SHAPES = [(256, 64), (64, 64), (64, 32), (32, OUT)]

_NEFF_CACHE_DIR = os.path.expanduser("~/.cache/bass_neff_cache")


def _install_neff_cache():
    """Cache walrus-compiled NEFFs on disk keyed by BIR hash."""
    if _BASS_CACHE.get('neff_patch'):
        return
    import hashlib, shutil, time as _t
    import concourse.bass_utils as bu
    import concourse.bass2jax as b2j
    orig = bu.compile_bir_kernel

    def cached(bir_json, tmpdir, neff_name="file.neff"):
        if isinstance(bir_json, str):
            bir_json = bir_json.encode()
        key = hashlib.sha256(bir_json).hexdigest()
        path = os.path.join(_NEFF_CACHE_DIR, key + ".neff")
        dst = os.path.join(tmpdir, neff_name)
        if os.path.exists(path):
            shutil.copyfile(path, dst)
            print(f"[kernel] neff cache hit {key[:12]}", file=sys.stderr)
            return dst
        t0 = _t.time()
        out = orig(bir_json, tmpdir, neff_name)
        print(f"[kernel] walrus compile: {_t.time()-t0:.1f}s",
              file=sys.stderr)
        try:
            os.makedirs(_NEFF_CACHE_DIR, exist_ok=True)
            tmp = path + ".tmp"
            shutil.copyfile(out, tmp)
            os.replace(tmp, path)
        except OSError:
            pass
        return out

    bu.compile_bir_kernel = cached
    b2j.compile_bir_kernel = cached
    _BASS_CACHE['neff_patch'] = True


def _build_bass(tcounts=None, debug=False, no_gather=False):
    if tcounts is None:
        tcounts = [[TMAX] * NBLK for _ in range(3)]
    import concourse.bass as bass
    import concourse.bacc as bacc
    import concourse.mybir as mybir
    from concourse.tile import TileContext
    from concourse.masks import make_identity
    AF = mybir.ActivationFunctionType
    ALU = mybir.AluOpType
    dt = mybir.dt

    nc = bacc.Bacc("TRN2", target_bir_lowering=False)
    dbg = {}
    if debug:
        dbg['xp0'] = nc.dram_tensor("dxp0", [256, 256], dt.float32,
                                    kind="ExternalOutput")
        dbg['xcA'] = nc.dram_tensor("dxcA", [128, NP_PAD], dt.float32,
                                    kind="ExternalOutput")
        dbg['xcB'] = nc.dram_tensor("dxcB", [128, NP_PAD], dt.float32,
                                    kind="ExternalOutput")
        for li, (fp_, o_) in enumerate(LAYER_SHAPES):
            dbg[f'h{li}'] = nc.dram_tensor(f"dh{li}", [o_, CH], dt.float32,
                                           kind="ExternalOutput")
        dbg['tsl0'] = nc.dram_tensor("dtsl0", [128, CH], dt.float32,
                                     kind="ExternalOutput")
        dbg['tp30'] = nc.dram_tensor("dtp30", [128, CH], dt.float32,
                                     kind="ExternalOutput")

    x_in = nc.dram_tensor("x", [N, IN], dt.float32, kind="ExternalInput")
    wt_in = nc.dram_tensor("wt", [IN, 3 * 256], dt.float32,
                           kind="ExternalInput")
    gbias_in = nc.dram_tensor("gbias", [128, 2], dt.float32,
                              kind="ExternalInput")
    w3_in = nc.dram_tensor("w3b", [3, 1], dt.float32, kind="ExternalInput")
    ei_in, ee_in = [], []
    for b in range(3):
        ei_in.append(nc.dram_tensor(f"ei{b}", [NBLK, 128, 2 * TMAX], dt.int32,
                                    kind="ExternalInput"))
        ee_in.append(nc.dram_tensor(f"ee{b}", [NBLK, 128, TMAX * 4],
                                    dt.float32, kind="ExternalInput"))
    lws, biases = [], []
    for li, (fin_pad, o) in enumerate(LAYER_SHAPES):
        rows = 128 if fin_pad >= 128 else fin_pad
        nft = fin_pad // rows
        nblk = nft * (1 + 3 * NM)
        lws.append(nc.dram_tensor(f"lw{li}", [rows, nblk * o], dt.float32,
                                  kind="ExternalInput"))
        biases.append(nc.dram_tensor(f"bias{li}", [o, 1], dt.float32,
                                     kind="ExternalInput"))
    y = nc.dram_tensor("y", [1, NP_PAD], dt.float32, kind="ExternalOutput")

    with TileContext(nc) as tc:
        with tc.tile_pool(name="persist", bufs=1) as pp, \
             tc.tile_pool(name="dram", bufs=1, space="DRAM") as dp:
            # persistent tiles
            ident = pp.tile([128, 128], dt.float32, name="ident")
            make_identity(nc, ident[:, :])
            iota_t = pp.tile([128, 128], dt.int32, name="iota_t")
            nc.gpsimd.iota(iota_t[:, :], pattern=[[1, 128]], base=0,
                           channel_multiplier=0)
            wt_sb = pp.tile([IN, 3 * 256], dt.float32, name="wt_sb")
            nc.sync.dma_start(wt_sb[:, :], wt_in[:, :])
            gbias_sb = pp.tile([128, 2], dt.float32, name="gbias_sb")
            nc.sync.dma_start(gbias_sb[:, :], gbias_in[:, :])
            w3_sb = pp.tile([3, 1], dt.float32, name="w3_sb")
            nc.sync.dma_start(w3_sb[:, :], w3_in[:, :])
            bconst = pp.tile([128, NM], dt.float32, name="bconst")
            for m in range(NM):
                nc.gpsimd.memset(bconst[:, m:m + 1], float(UBI - m))
            lw_sb, bias_sb = [], []
            for li, (fin_pad, o) in enumerate(LAYER_SHAPES):
                t = pp.tile(list(lws[li].shape), dt.float32, name=f"lwsb{li}")
                nc.sync.dma_start(t[:, :], lws[li][:, :])
                lw_sb.append(t)
                bt = pp.tile([o, 1], dt.float32, name=f"biassb{li}")
                nc.sync.dma_start(bt[:, :], biases[li][:, :])
                bias_sb.append(bt)
            # output feature tiles: xcA rows 0:128, xcB rows 128:256
            xcA = pp.tile([128, NP_PAD], dt.float32, name="xcA")
            xcB = pp.tile([128, NP_PAD], dt.float32, name="xcB")
            nc.vector.memset(xcB[64:128, :], 0.0)
            nc.vector.tensor_copy(
                xcB[64:67, :], w3_sb[:, 0:1].to_broadcast([3, NP_PAD]))

            # per-branch xp tables in DRAM
            xp_t = [dp.tile([N, 256], dt.float32, name=f"xp{b}", tag=f"xp{b}")
                    for b in range(3)]

            # ---- phase A: xp[b] = x @ W_b.T, written as [N, 256] rows ----
            with tc.tile_pool(name="pa", bufs=3) as pa, \
                 tc.tile_pool(name="pap", bufs=2, space="PSUM") as pap, \
                 tc.tile_pool(name="pap2", bufs=2, space="PSUM") as pap2:
                for c in range(NXCH):
                    r0 = c * 128
                    rows = min(128, N - r0)
                    xtile = pa.tile([128, IN], dt.float32, name="xtile")
                    nc.sync.dma_start(xtile[:rows, :], x_in[r0:r0 + rows, :])
                    tp = pap.tile([128, 128], dt.float32, name="tp")
                    nc.tensor.transpose(out=tp[:, :], in_=xtile[:, :],
                                        identity=ident[:, :])
                    xT = pa.tile([128, 128], dt.float32, name="xT")
                    nc.vector.tensor_copy(xT[:, :], tp[:, :])
                    mm1 = pap2.tile([128, 512], dt.float32, name="mm1")
                    nc.tensor.matmul(mm1[:rows, :], xT[:, :rows],
                                     wt_sb[:, 0:512], start=True, stop=True)
                    mm2 = pap2.tile([128, 256], dt.float32, name="mm2")
                    nc.tensor.matmul(mm2[:rows, :], xT[:, :rows],
                                     wt_sb[:, 512:768], start=True, stop=True)
                    xps1 = pa.tile([128, 512], dt.float32, name="xps1")
                    nc.scalar.activation(xps1[:rows, :], mm1[:rows, :],
                                         AF.Copy)
                    xps2 = pa.tile([128, 256], dt.float32, name="xps2")
                    nc.scalar.activation(xps2[:rows, :], mm2[:rows, :],
                                         AF.Copy)
                    nc.sync.dma_start(xp_t[0][r0:r0 + rows, :],
                                      xps1[:rows, 0:256])
                    nc.sync.dma_start(xp_t[1][r0:r0 + rows, :],
                                      xps1[:rows, 256:512])
                    nc.sync.dma_start(xp_t[2][r0:r0 + rows, :],
                                      xps2[:rows, :])

            # ---- phase B: GAT aggregation per branch/block ----
            with tc.tile_pool(name="pb", bufs=2) as pb, \
                 tc.tile_pool(name="pbg", bufs=2) as pbg, \
                 tc.tile_pool(name="pbp", bufs=2, space="PSUM") as pbp, \
                 tc.tile_pool(name="pbp2", bufs=2, space="PSUM") as pbp2:
                for b in range(3):
                    for blk in range(NBLK):
                        Tb = tcounts[b][blk]
                        eint = pb.tile([128, 2 * TMAX], dt.int32, name="eint")
                        nc.sync.dma_start(eint[:, :], ei_in[b][blk][:, :])
                        eex = pb.tile([128, TMAX, 4], dt.float32, name="eex")
                        nc.sync.dma_start(eex[:, :, :], ee_in[b][blk][:, :])
                        ps = pbp.tile([128, 260], dt.float32, name="ps")
                        xg = pbg.tile([128, TMAX, 256], dt.float32,
                                      name="xg")
                        if no_gather:
                            nc.vector.memset(xg[:, :Tb, :], 0.5)
                        else:
                            for t in range(Tb):
                                nc.gpsimd.indirect_dma_start(
                                    out=xg[:, t, :],
                                    out_offset=None,
                                    in_=xp_t[b][:, :],
                                    in_offset=bass.IndirectOffsetOnAxis(
                                        ap=eint[:, t:t + 1], axis=0))
                        S = pbg.tile([128, TMAX, 128], dt.float32,
                                     name="S")
                        nc.vector.tensor_tensor(
                            out=S[:, :Tb, :],
                            in0=eint[:, TMAX:TMAX + Tb, None]
                            .to_broadcast([128, Tb, 128]),
                            in1=iota_t[:, None, :]
                            .to_broadcast([128, Tb, 128]),
                            op=ALU.is_equal)
                        wxe = pbg.tile([128, TMAX, 260], dt.float32,
                                       name="wxe")
                        for h in range(H):
                            nc.vector.tensor_tensor(
                                out=wxe[:, :Tb, h * 64:(h + 1) * 64],
                                in0=xg[:, :Tb, h * 64:(h + 1) * 64],
                                in1=eex[:, :Tb, h:h + 1]
                                .to_broadcast([128, Tb, 64]),
                                op=ALU.mult)
                        nc.vector.tensor_copy(
                            wxe[:, :Tb, 256:260], eex[:, :Tb, :])
                        for t in range(Tb):
                            nc.tensor.matmul(
                                ps[:, :], S[:, t, :], wxe[:, t, :],
                                start=(t == 0), stop=(t == Tb - 1))
                        rec = pb.tile([128, 4], dt.float32, name="rec")
                        nc.vector.reciprocal(rec[:, :], ps[:, 256:260])
                        on = pb.tile([128, 64], dt.float32, name="on")
                        tmp = pb.tile([128, 64], dt.float32, name="tmp")
                        nc.vector.tensor_scalar(
                            on[:, :], ps[:, 0:64], rec[:, 0:1], 0.25,
                            op0=ALU.mult, op1=ALU.mult)
                        for h in range(1, H):
                            nc.vector.tensor_scalar(
                                tmp[:, :], ps[:, h * 64:(h + 1) * 64],
                                rec[:, h:h + 1], 0.25,
                                op0=ALU.mult, op1=ALU.mult)
                            nc.vector.tensor_add(on[:, :], on[:, :],
                                                 tmp[:, :])
                        tps = pbp2.tile([64, 128], dt.float32, name="tps")
                        nc.tensor.transpose(out=tps[:, :], in_=on[:, :],
                                            identity=ident[:, :])
                        dst_tile = xcA if b < 2 else xcB
                        prow = (b % 2) * 64
                        bias_ap = gbias_sb[prow:prow + 64, b // 2:b // 2 + 1]
                        nc.scalar.activation(
                            dst_tile[prow:prow + 64,
                                     blk * 128:(blk + 1) * 128],
                            tps[:, :], AF.Identity, bias=bias_ap)

            if debug:
                nc.sync.dma_start(dbg['xp0'][:, :], xp_t[0][0:256, :])
                nc.sync.dma_start(dbg['xcA'][:, :], xcA[:, :])
                nc.sync.dma_start(dbg['xcB'][:, :], xcB[:, :])

            # ---- phase C: KAN layers over node chunks ----
            with tc.tile_pool(name="tpool", bufs=3) as tpool, \
                 tc.tile_pool(name="opool", bufs=2) as opool, \
                 tc.tile_pool(name="ppool", bufs=2, space="PSUM") as ppool:
                for c in range(NCH):
                    cur = [xcA[:, c * CH:(c + 1) * CH],
                           xcB[:, c * CH:(c + 1) * CH]]
                    for li, (fin_pad, o) in enumerate(LAYER_SHAPES):
                        rows = 128 if fin_pad >= 128 else fin_pad
                        nft = fin_pad // rows
                        nblk = nft * (1 + 3 * NM)
                        ps = ppool.tile([o, CH], dt.float32, name="kps",
                                        tag="kps")
                        blk = 0
                        for f in range(nft):   # silu blocks
                            tsl = tpool.tile([rows, CH], dt.float32,
                                             name="tsl", tag="tsl")
                            nc.scalar.activation(tsl[:, :], cur[f][:rows, :],
                                                 AF.Silu)
                            if debug and c == 0 and li == 0 and f == 0:
                                nc.sync.dma_start(dbg['tsl0'][:, :],
                                                  tsl[:, :])
                            nc.tensor.matmul(
                                ps[:, :], lw_sb[li][:, blk * o:(blk + 1) * o],
                                tsl[:, :], start=(blk == 0),
                                stop=(blk == nblk - 1))
                            blk += 1
                        for f in range(nft):
                            for m in range(NM):
                                tv = tpool.tile([rows, CH], dt.float32,
                                                name="tv", tag="tv")
                                nc.scalar.activation(
                                    tv[:, :], cur[f][:rows, :], AF.Relu,
                                    bias=bconst[:rows, m:m + 1],
                                    scale=float(USC))
                                tp1 = tpool.tile([rows, CH], dt.float32,
                                                 name="tp1", tag="tp1")
                                nc.scalar.activation(
                                    tp1[:, :], tv[:, :], AF.Relu,
                                    bias=1.0, scale=-1.0)
                                tp2 = tpool.tile([rows, CH], dt.float32,
                                                 name="tp2", tag="tp2")
                                nc.scalar.activation(tp2[:, :], tp1[:, :],
                                                     AF.Square)
                                tp3 = tpool.tile([rows, CH], dt.float32,
                                                 name="tp3", tag="tp3")
                                nc.vector.tensor_mul(tp3[:, :], tp2[:, :],
                                                     tp1[:, :])
                                if debug and c == 0 and li == 0 and f == 0 \
                                        and m == 0:
                                    nc.sync.dma_start(dbg['tp30'][:, :],
                                                      tp3[:, :])
                                for t in (tp1, tp2, tp3):
                                    nc.tensor.matmul(
                                        ps[:, :],
                                        lw_sb[li][:, blk * o:(blk + 1) * o],
                                        t[:, :], start=(blk == 0),
                                        stop=(blk == nblk - 1))
                                    blk += 1
                        outt = opool.tile([o, CH], dt.float32, name="outt",
                                          tag=f"out{li}")
                        func = AF.Relu if li == 1 else AF.Identity
                        nc.scalar.activation(outt[:, :], ps[:, :], func,
                                             bias=bias_sb[li][:, 0:1])
                        if debug and c == 0:
                            nc.sync.dma_start(dbg[f'h{li}'][:, :],
                                              outt[:, :])
                        cur = [outt]
                    nc.sync.dma_start(y[0:1, c * CH:(c + 1) * CH],
                                      cur[0][0:1, :])
    nc.finalize()
    return nc


def _prep_branch(x, ei, W, a_src, a_dst):
    """Sort edges by dst, compute per-edge exp(leaky(score)), pack streams."""
    # attention projections: as_/ad_ = x @ A where A[:, h] = W_h.T @ a_h
    A = np.empty((IN, 2 * H), np.float32)
    for h in range(H):
        Wh = W[h * HID:(h + 1) * HID, :]          # [64, 128]
        A[:, h] = Wh.T @ a_src[h]
        A[:, H + h] = Wh.T @ a_dst[h]
    proj = x @ A                                   # [N, 8]
    as_, ad_ = proj[:, :H], proj[:, H:]

    loops = np.arange(N, dtype=np.int32)
    src_all = np.concatenate([ei[0].astype(np.int32), loops])
    dst_all = np.concatenate([ei[1].astype(np.int32), loops])
    order = np.argsort(dst_all, kind='stable')
    src_s = src_all[order]
    dst_s = dst_all[order]
    e = as_[src_s] + ad_[dst_s]
    e = np.where(e > 0, e, np.float32(0.2) * e)
    ex = np.exp(e, dtype=np.float32)               # [M, 4]

    M_ = dst_s.shape[0]
    core = dst_s // SH
    loc = dst_s - core * SH
    blk = loc >> 7
    dl = (loc & 127).astype(np.int32)
    gblk = core * NBLK + blk
    cnt = np.bincount(gblk, minlength=NC * NBLK)
    starts = np.zeros(NC * NBLK, np.int64)
    np.cumsum(cnt[:-1], out=starts[1:])
    rank = np.arange(M_, dtype=np.int64) - starts[gblk]

    # pad-node dummy self-edges (locals SH..NP_PAD-1, same for each core)
    pad_loc = np.arange(SH, NP_PAD, dtype=np.int64)
    pblk = pad_loc >> 7
    pdl = (pad_loc & 127).astype(np.int32)
    pseq = pad_loc - np.maximum(SH, pblk * 128)
    cnt2 = cnt.reshape(NC, NBLK)
    if (cnt2.max(axis=0) + np.bincount(pblk, minlength=NBLK)).max() \
            > TMAX * 128:
        raise RuntimeError("block capacity exceeded")

    ei_arr = np.zeros((NC, NBLK, 128, 2 * TMAX), np.int32)
    ei_arr[:, :, :, TMAX:] = 999
    ee_arr = np.zeros((NC, NBLK, 128, TMAX, 4), np.float32)
    p_ = (rank & 127).astype(np.int64)
    t_ = rank >> 7
    ei_arr[core, blk, p_, t_] = src_s
    ei_arr[core, blk, p_, TMAX + t_] = dl
    ee_arr[core, blk, p_, t_] = ex
    for c in range(NC):
        prank = cnt2[c, pblk] + pseq
        pp_ = prank & 127
        pt_ = prank >> 7
        ei_arr[c, pblk, pp_, pt_] = 0
        ei_arr[c, pblk, pp_, TMAX + pt_] = pdl
        ee_arr[c, pblk, pp_, pt_] = 1.0
    # per-block tile count: max over cores (SPMD shares one program)
    tot = cnt2 + np.bincount(pblk, minlength=NBLK)[None, :]
    tcnt = [int(v) for v in np.maximum(1, -(-tot.max(axis=0) // 128))]
    return ei_arr, ee_arr.reshape(NC, NBLK, 128, TMAX * 4), tcnt


def kernel(**inputs):
    import time as _time
    _t0 = _time.time()

    def _lap(msg):
        print(f"[kernel] {msg}: {_time.time() - _t0:.2f}s", file=sys.stderr)

    ins = {k: np.asarray(v) for k, v in inputs.items()}
    x = np.ascontiguousarray(ins['x'].astype(np.float32))

    al = np.array([ins['alpha_adj'], ins['alpha_od'], ins['alpha_od_t']],
                  np.float64)
    w3 = np.exp(al - al.max()); w3 = (w3 / w3.sum()).astype(np.float32)

    weights = [(ins['fk0_base'], ins['fk0_spline'], ins['fk0_scaler']),
               (ins['fk1_base'], ins['fk1_spline'], ins['fk1_scaler']),
               (ins['k0_base'], ins['k0_spline'], ins['k0_scaler']),
               (ins['k1_base'], ins['k1_spline'], ins['k1_scaler'])]
    branches = [('adj', 'edge_index_adj'), ('od', 'edge_index_od'),
                ('odt', 'edge_index_od_t')]

    try:
        M, fit_err = _fit_M()
        if fit_err > 1e-8:
            raise RuntimeError(f"phi basis fit err {fit_err}")
        folded = []
        for (bw, sw, sc), (fin_pad, o) in zip(weights, LAYER_SHAPES):
            folded.append(_fold_layer(bw.astype(np.float32),
                                      sw.astype(np.float32),
                                      sc.astype(np.float32), M, fin_pad))
        _lap("fold")

        wt = np.concatenate(
            [np.ascontiguousarray(ins[p + '_W'].astype(np.float32).T)
             for p, _ in branches], axis=1)           # [128, 768]
        gbias = np.zeros((128, 2), np.float32)
        gbias[0:64, 0] = ins['adj_bias']
        gbias[64:128, 0] = ins['od_bias']
        gbias[0:64, 1] = ins['odt_bias']

        eis, ees, tcounts = [], [], []
        for p, ek in branches:
            ei_arr, ee_arr, tcnt = _prep_branch(
                x, ins[ek], ins[p + '_W'].astype(np.float32),
                ins[p + '_att_src'].astype(np.float32),
                ins[p + '_att_dst'].astype(np.float32))
            eis.append(ei_arr)
            ees.append(ee_arr)
            tcounts.append(tcnt)
        _lap("edge prep")

        ckey = ('nc', tuple(tuple(t) for t in tcounts))
        if ckey not in _BASS_CACHE:
            _BASS_CACHE[ckey] = _build_bass(tcounts)
        nc = _BASS_CACHE[ckey]
        _lap("build_bass")

        base = {"x": x, "wt": wt, "gbias": gbias,
                "w3b": w3.reshape(3, 1)}
        for li, (lw, bvec) in enumerate(folded):
            nb_, r_, o_ = lw.shape
            base[f"lw{li}"] = np.ascontiguousarray(
                lw.transpose(1, 0, 2).reshape(r_, nb_ * o_))
            base[f"bias{li}"] = bvec.reshape(-1, 1)
        in_maps = []
        for i in range(NC):
            m_ = dict(base)
            for b in range(3):
                m_[f"ei{b}"] = np.ascontiguousarray(eis[b][i])
                m_[f"ee{b}"] = np.ascontiguousarray(ees[b][i])
            in_maps.append(m_)
        from concourse.bass_utils import run_bass_kernel_spmd
        _install_neff_cache()
        _lap("in_maps ready")
        res = run_bass_kernel_spmd(nc, in_maps, core_ids=list(range(NC)))
        _lap("spmd done")
        y = np.concatenate([res.results[i]["y"][0, :SH] for i in range(NC)])
        y = y.reshape(N, OUT).astype(np.float32)
        if not np.isfinite(y).all():
            raise RuntimeError("non-finite output from device")
        return y
    except Exception as exc:
        print(f"[kernel] bass path failed ({exc}); host fallback",
              file=sys.stderr)
        outs = []
        for p, ek in branches:
            outs.append(_host_gat(x, ins[ek],
                                  ins[p + '_W'].astype(np.float32),
                                  ins[p + '_att_src'], ins[p + '_att_dst'],
                                  ins[p + '_bias']))
        xc = np.concatenate(outs + [np.broadcast_to(w3, (N, 3))], axis=1)
        return _host_kan(xc, weights).reshape(N, OUT).astype(np.float32)

